# revision 43
# baseline (speedup 1.0000x reference)
"""Trainium2 Bass kernel: sliding-window causal attention with ALiBi.

Problem: B=2, T=2048, HID=2048, NH=32, DH=64, window=1024, f32.
  q,k,v = hs@Wq/sqrt(DH), hs@Wk, hs@Wv  (per-head views)
  out   = softmax(mask(q k^T + alibi)) v  @ Wo

Sharding (8 cores): batch-split x head-split. Cores 0-3 own batch 0,
cores 4-7 batch 1; within a 4-core group, core (rank r) owns the 8 heads
{r + 4*lh}, so each core reads only its batch's activations and a 512-col
slice of each projection matrix. AllGather over the 4-core group
reassembles the head dim for the output projection.

Key implementation points:
  - All matmul operands bf16 (q/k/v/o projections, QK^T, PV); f32 PSUM.
  - Everything transposed ([feature, token]): contraction lands on SBUF
    partitions; q/k/v stay SBUF-resident (no DRAM round trip).
  - ALiBi folded into QK as 3 extra contraction channels (k-side values
    range-reduced per 128-tile so bf16 operand error stays proportional
    to the bias itself; the per-q channel cancels in softmax). K=67.
  - Scores computed transposed sT[k,q]; softmax along k needs no row-max
    (logits bounded) and the denominator falls out of PV via a ones
    column appended to V.
  - Per-stripe normalization: denom row (staged to SBUF partition 0 —
    reciprocal_approx_fast misreads PSUM/offset-base inputs) ->
    reciprocal_approx_fast -> PE rank-1 broadcast (ones[1,64]^T @
    inv[1,256]) -> DVE multiply.
  - Sliding-window + ALiBi-decay truncation picks k-tiles per (slot,
    stripe); identical structure on every core (SPMD). Diag/edge masks
    are additive f32 on the scores pre-exp (post-exp multiplicative
    masking NaNs on the +inf diag overflow); fully-invalid halves are
    memset after exp.
  - Emission: per token tile, projections (A) then a 3-deep
    software-pipelined wave of 16 attention stripes (so the PE always
    has independent QK/PV work while another stripe's mask->exp chain
    resolves -- keeps the HAM clock warm), then that token quarter's
    AllGather; the 4 output-projection tiles (D) trail, hiding all but
    the last AllGather.
"""

import math
import sys

sys.path.insert(0, "/opt/trn_rl_repo")

import numpy as np
import ml_dtypes

import concourse.mybir as mybir
import concourse.tile as tile
from concourse import bacc
from concourse.bass_utils import run_bass_kernel_spmd

F32 = mybir.dt.float32
F32R = mybir.dt.float32r
BF16 = mybir.dt.bfloat16
BF = ml_dtypes.bfloat16

B, T, HID, NH, DH = 2, 2048, 2048, 32, 64
WIN = 1024
N_CORES = 8
NGRP = 4                      # cores per replica group (one batch)
HPC = NH // NGRP              # heads per core = 8
CW = HPC * DH                 # per-core feature slice = 512
NSTRIPE = T // 256            # 8 q-stripes
NAUG = 3
KC = DH + NAUG                # QK contraction = 67
MARGIN = 30.0
GROUPS = [[0, 1, 2, 3], [4, 5, 6, 7]]


def _slopes():
    start = 2 ** (-(2 ** -(math.log2(NH) - 3)))
    return [start ** (i + 1) for i in range(NH)]


def _slot_kts(lh, s):
    """k-tiles attended by q-stripe s for head-slot lh (SPMD-shared)."""
    sl = _slopes()[4 * lh + 3]  # smallest slope (widest window) in the slot
    return [kt for kt in range(max(0, 2 * s - 8), 2 * s + 2)
            if sl * max(0, 128 * (2 * s - kt) - 127) < MARGIN]


_NC_CACHE = {}
DEBUG_TAPS = False


def build_nc():
    key = ("nc", DEBUG_TAPS)
    if key in _NC_CACHE:
        return _NC_CACHE[key]
    nc = bacc.Bacc(None, target_bir_lowering=False, debug=False)

    xT = nc.declare_dram_parameter("xT", [HID, T], BF16, isOutput=False)
    wq = nc.declare_dram_parameter("wq", [HID, CW], BF16, isOutput=False)
    wk = nc.declare_dram_parameter("wk", [HID, CW], BF16, isOutput=False)
    wv = nc.declare_dram_parameter("wv", [HID, CW], BF16, isOutput=False)
    wo = nc.declare_dram_parameter("wo", [HID, CW], BF16, isOutput=False)
    caug = nc.declare_dram_parameter("caug", [2, HPC, NAUG, T], BF16,
                                     isOutput=False)
    msk = nc.declare_dram_parameter("msk", [2, 128, 128], F32, isOutput=False)
    outT = nc.declare_dram_parameter("outT", [CW, T], F32, isOutput=True)
    if DEBUG_TAPS:
        dq = nc.declare_dram_parameter("dq", [CW, T], BF16, isOutput=True)
        dk = nc.declare_dram_parameter("dk", [CW, T], BF16, isOutput=True)
        dv = nc.declare_dram_parameter("dv", [128, 16 * HPC * (DH + 1)], BF16,
                                       isOutput=True)
        dao = nc.declare_dram_parameter("dao", [NGRP * 128, T], BF16,
                                        isOutput=True)
        dag = nc.declare_dram_parameter("dag", [NGRP * CW, T], BF16,
                                        isOutput=True)

    with tile.TileContext(nc) as tc:
        with tc.tile_pool(name="dram", bufs=1, space="DRAM") as dram, \
             tc.tile_pool(name="constp", bufs=1) as constp, \
             tc.tile_pool(name="wp", bufs=48) as wp, \
             tc.tile_pool(name="wop", bufs=16) as wop, \
             tc.tile_pool(name="xtp", bufs=18) as xtp, \
             tc.tile_pool(name="kqp", bufs=1) as kqp, \
             tc.tile_pool(name="vtp", bufs=1) as vtp, \
             tc.tile_pool(name="aop", bufs=1) as aop, \
             tc.tile_pool(name="ppool", bufs=4) as ppool, \
             tc.tile_pool(name="evp", bufs=2) as evp, \
             tc.tile_pool(name="invp", bufs=2) as invp, \
             tc.tile_pool(name="brsp", bufs=2) as brsp, \
             tc.tile_pool(name="stp", bufs=5, space="PSUM") as stp, \
             tc.tile_pool(name="pvp", bufs=3, space="PSUM") as pvp:

            cc = [dram.tile([CW, 512], BF16, name=f"cc{i}") for i in range(4)]
            ag = [dram.tile([NGRP * CW, 512], BF16, name=f"ag{i}")
                  for i in range(4)]

            mask_diag = constp.tile([128, 128], F32)
            mask_edge = constp.tile([128, 128], F32)
            nc.gpsimd.dma_start(mask_diag[:], msk[0])
            nc.gpsimd.dma_start(mask_edge[:], msk[1])
            ones64 = constp.tile([1, 64], BF16)
            nc.vector.memset(ones64[:], 1.0)

            # persistent SBUF tensors
            kaug = [kqp.tile([KC, T], BF16, name=f"kaug{h}") for h in range(HPC)]
            qaug = [kqp.tile([KC, T], BF16, name=f"qaug{h}") for h in range(HPC)]
            vt = vtp.tile([128, 16, HPC, DH + 1], BF16, name="vt")
            ao = [aop.tile([128, T], BF16, name=f"ao{i}") for i in range(NGRP)]

            nc.vector.memset(vt[:, :, :, DH:DH + 1], 1.0)
            for lh in range(HPC):
                nc.gpsimd.dma_start(kaug[lh][DH:KC, :], caug[0, lh])
                nc.gpsimd.dma_start(qaug[lh][DH:KC, :], caug[1, lh])

            # weights
            wq_sb, wk_sb, wv_sb, wo_sb = [], [], [], []
            for kt in range(16):
                t_ = wp.tile([128, CW], BF16, name=f"wq_{kt}", tag="w")
                nc.scalar.dma_start(t_[:], wq[kt * 128:(kt + 1) * 128, :])
                wq_sb.append(t_)

            def load_w(dst, src, pfx, eng, pool, tag):
                for kt in range(16):
                    t_ = pool.tile([128, CW], BF16, name=f"{pfx}_{kt}", tag=tag)
                    eng.dma_start(t_[:], src[kt * 128:(kt + 1) * 128, :])
                    dst.append(t_)

            # ---------- phase A: projections for one 512-token tile ----------
            def a_emit(tokt):
                t0 = tokt * 512
                xts = []
                for kt in range(16):
                    eng = nc.sync if kt % 2 == 0 else nc.scalar
                    xt_t = xtp.tile([128, 512], BF16, name=f"xt_{tokt}_{kt}",
                                    tag="xt")
                    eng.dma_start(xt_t[:], xT[kt * 128:(kt + 1) * 128,
                                              t0:t0 + 512])
                    xts.append(xt_t)
                if tokt == 0:
                    load_w(wk_sb, wk, "wk", nc.sync, wp, "w")
                    load_w(wv_sb, wv, "wv", nc.gpsimd, wp, "w")
                    load_w(wo_sb, wo, "wo", nc.gpsimd, wop, "wo")

                # q, k -> [feat, tok] slices of qaug/kaug (2 heads per M-tile)
                for pi, (w_sb, dst) in enumerate(((wq_sb, qaug), (wk_sb, kaug))):
                    for mt in range(4):
                        ps = stp.tile([128, 512], F32, tag="st",
                                      name=f"ps{pi}_{tokt}_{mt}")
                        for kt in range(16):
                            nc.tensor.matmul(
                                ps[:], w_sb[kt][:, mt * 128:(mt + 1) * 128],
                                xts[kt][:], start=(kt == 0), stop=(kt == 15))
                        for hh in range(2):
                            lh = 2 * mt + hh
                            eng = nc.vector if (mt + hh) % 2 == 0 else nc.scalar
                            if eng is nc.vector:
                                nc.vector.tensor_copy(
                                    dst[lh][0:DH, t0:t0 + 512],
                                    ps[hh * DH:(hh + 1) * DH, :])
                            else:
                                nc.scalar.activation(
                                    dst[lh][0:DH, t0:t0 + 512],
                                    ps[hh * DH:(hh + 1) * DH, :],
                                    mybir.ActivationFunctionType.Copy)

                # v -> [tok, feat] tiles of vt
                for sub in range(4):
                    gkt = 4 * tokt + sub
                    psv = stp.tile([128, CW], F32, tag="st",
                                   name=f"psv_{tokt}_{sub}")
                    for kt in range(16):
                        nc.tensor.matmul(
                            psv[:], xts[kt][:, sub * 128:(sub + 1) * 128],
                            wv_sb[kt][:], start=(kt == 0), stop=(kt == 15))
                    nc.vector.tensor_copy(
                        vt[:, gkt, :, 0:DH],
                        psv[:].rearrange("p (h d) -> p h d", h=HPC))

            # ---------- phase B: one stripe of one head-slot ----------------
            def stripe_units(lh, s):
                """Generator yielding emission units for one stripe."""
                q0 = s * 256
                kts = _slot_kts(lh, s)
                nk = len(kts)
                groups = [kts[i:i + 2] for i in range(0, nk, 2)]
                tiles = {}
                st_tiles = {}

                def alloc():
                    tiles["p"] = ppool.tile([128, 2560], BF16, tag="p",
                                            name=f"p_{lh}_{s}")
                    tiles["pv0"] = pvp.tile([DH + 1, 256], F32, tag="pv",
                                            name=f"pv_{lh}_{s}")

                def qk_group(gi):
                    if gi == 0:
                        alloc()
                    g = groups[gi]
                    stt = stp.tile([128, 512], F32, tag="st",
                                   name=f"st_{lh}_{s}_{gi}")
                    st_tiles[gi] = stt
                    for j, kt in enumerate(g):
                        nc.tensor.matmul(
                            stt[:, j * 256:j * 256 + 256],
                            kaug[lh][:, kt * 128:(kt + 1) * 128],
                            qaug[lh][:, q0:q0 + 256], start=True, stop=True)

                def post_group(gi):
                    g = groups[gi]
                    stt = st_tiles.pop(gi)
                    p = tiles["p"]
                    ki0 = 2 * gi
                    for j, kt in enumerate(g):
                        for hh in range(2):
                            rs = 2 * s + hh - kt
                            c0 = j * 256 + hh * 128
                            if rs == 0:
                                nc.vector.tensor_tensor(
                                    stt[:, c0:c0 + 128], stt[:, c0:c0 + 128],
                                    mask_diag[:], mybir.AluOpType.add)
                            elif rs == 8:
                                nc.vector.tensor_tensor(
                                    stt[:, c0:c0 + 128], stt[:, c0:c0 + 128],
                                    mask_edge[:], mybir.AluOpType.add)
                    nc.scalar.activation(
                        p[:, ki0 * 256:(ki0 + len(g)) * 256],
                        stt[:, 0:len(g) * 256],
                        mybir.ActivationFunctionType.Exp)
                    for j, kt in enumerate(g):
                        ki = ki0 + j
                        for hh in range(2):
                            rs = 2 * s + hh - kt
                            if rs < 0 or rs > 8:
                                c0 = ki * 256 + hh * 128
                                nc.vector.memset(p[:, c0:c0 + 128], 0.0)

                def pv(ki):
                    kt = kts[ki]
                    nc.tensor.matmul(
                        tiles["pv0"][:], vt[:, kt, lh, :],
                        tiles["p"][:, ki * 256:(ki + 1) * 256],
                        start=(ki == 0), stop=(ki == nk - 1))

                def pvs_for(gi):
                    def emit():
                        post_group(gi)
                        pv(2 * gi)
                        if 2 * gi + 1 < nk:
                            pv(2 * gi + 1)
                    return emit

                def normalize():
                    # recip_approx_fast misreads PSUM/offset-base inputs;
                    # stage the denom row in SBUF at partition 0 first.
                    pv0 = tiles["pv0"]
                    dens = invp.tile([1, 256], F32, tag="nrm",
                                     name=f"dens_{lh}_{s}")
                    nc.vector.tensor_copy(dens[:], pv0[DH:DH + 1, :])
                    inv = invp.tile([1, 256], F32, tag="nrm",
                                    name=f"inv_{lh}_{s}")
                    nc.vector.reciprocal_approx_fast(inv[:], dens[:])
                    invb = invp.tile([1, 256], BF16, tag="nrm",
                                     name=f"invb_{lh}_{s}")
                    nc.vector.tensor_copy(invb[:], inv[:])
                    br = stp.tile([DH, 256], F32, tag="st",
                                  name=f"br_{lh}_{s}")
                    nc.tensor.matmul(br[:], ones64[:], invb[:],
                                     start=True, stop=True)
                    brs = brsp.tile([DH, 256], BF16, tag="brs",
                                    name=f"brs_{lh}_{s}")
                    nc.vector.tensor_copy(brs[:], br[:])
                    nc.vector.tensor_tensor(
                        ao[lh // 2][(lh % 2) * DH:(lh % 2) * DH + DH,
                                    q0:q0 + 256],
                        pv0[0:DH, :], brs[:], mybir.AluOpType.mult)

                ng = len(groups)
                yield lambda: qk_group(0)
                for gi in range(1, ng):
                    yield lambda gi=gi: qk_group(gi)
                    yield pvs_for(gi - 1)
                yield pvs_for(ng - 1)
                yield normalize

            def b_chunk(tokt, extra_gens=()):
                # software-pipeline stripes 3-deep so the PE always has
                # independent QK work while another stripe's exp chain runs
                gens = [stripe_units(lh, s)
                        for s in (2 * tokt, 2 * tokt + 1)
                        for lh in range(HPC)]
                gens.extend(extra_gens)
                W = 3
                active, idx = [], 0
                while active or idx < len(gens):
                    while len(active) < W and idx < len(gens):
                        active.append(gens[idx])
                        idx += 1
                    for g in list(active):
                        try:
                            next(g)()
                        except StopIteration:
                            active.remove(g)

            def cc_dma(quarter):
                for i in range(NGRP):
                    nc.sync.dma_start(
                        cc[quarter][i * 128:(i + 1) * 128, :],
                        ao[i][:, quarter * 512:(quarter + 1) * 512])

            def allgather(quarter):
                nc.gpsimd.collective_compute(
                    "AllGather", mybir.AluOpType.bypass,
                    replica_groups=GROUPS,
                    ins=[cc[quarter][:].opt()], outs=[ag[quarter][:].opt()])

            # ---------- phase D: output projection for one 512-token tile ----
            def d_units(tokt, late):
                """Generator of emission units (dma, then one MM-group each).
                late=False keeps the sync queue free for the cc DMAs."""
                t0 = tokt * 512
                ats = []

                def dma():
                    for kt in range(16):
                        if late:
                            eng = nc.sync if kt % 2 == 0 else nc.scalar
                        else:
                            eng = nc.gpsimd if kt % 2 == 0 else nc.scalar
                        at = wp.tile([128, 512], BF16, name=f"agt_{tokt}_{kt}",
                                     tag="w")
                        eng.dma_start(at[:],
                                      ag[tokt][kt * 128:(kt + 1) * 128, :])
                        ats.append(at)

                def mm(mt):
                    ps = stp.tile([128, 512], F32, tag="st",
                                  name=f"psD_{tokt}_{mt}")
                    for kt in range(16):
                        nc.tensor.matmul(
                            ps[:], wo_sb[kt][:, mt * 128:(mt + 1) * 128],
                            ats[kt][:], start=(kt == 0), stop=(kt == 15))
                    ev = evp.tile([128, 512], F32, tag="ev",
                                  name=f"ev_{tokt}_{mt}")
                    nc.vector.tensor_copy(ev[:], ps[:])
                    nc.scalar.dma_start(
                        outT[mt * 128:(mt + 1) * 128, t0:t0 + 512], ev[:])

                yield dma
                for mt in range(4):
                    yield lambda mt=mt: mm(mt)

            # ---------------- emission schedule ----------------
            for tokt in range(4):
                a_emit(tokt)
                b_chunk(tokt)
                cc_dma(tokt)
                allgather(tokt)
            for tokt in range(4):
                for u in d_units(tokt, True):
                    u()
            if DEBUG_TAPS:
                for lh in range(HPC):
                    nc.sync.dma_start(dq[lh * DH:(lh + 1) * DH, :],
                                      qaug[lh][0:DH, :])
                    nc.sync.dma_start(dk[lh * DH:(lh + 1) * DH, :],
                                      kaug[lh][0:DH, :])
                nc.sync.dma_start(
                    dv[:, :], vt[:].rearrange("p a b c -> p (a b c)"))
                for i in range(NGRP):
                    nc.sync.dma_start(dao[i * 128:(i + 1) * 128, :], ao[i][:])
                for qtr in range(4):
                    nc.sync.dma_start(
                        dag[:, qtr * 512:(qtr + 1) * 512], ag[qtr][:])

    nc.finalize()
    _NC_CACHE["nc"] = nc
    return nc


def make_in_maps(hidden_states, Wq, Wk, Wv, Wo):
    slopes = _slopes()
    hs = np.asarray(hidden_states, dtype=np.float32)

    tok = np.arange(T, dtype=np.float32)
    idx = np.arange(128)
    # additive pre-exp masks: diag keeps q>=k, edge keeps q<k
    NEG = np.float32(-30000.0)
    mask_diag = np.where(idx[None, :] >= idx[:, None], 0.0, NEG)
    mask_edge = np.where(idx[None, :] < idx[:, None], 0.0, NEG)
    msk = np.stack([mask_diag, mask_edge]).astype(np.float32)

    wq_s = np.asarray(Wq, np.float32) / math.sqrt(DH)
    Wk_, Wv_, Wo_ = (np.asarray(w, np.float32) for w in (Wk, Wv, Wo))

    # wo rows ordered to match the AllGather layout (rank r, slot lh, d)
    perm = np.empty(HID, np.int64)
    for r in range(NGRP):
        for lh in range(HPC):
            g = r + NGRP * lh
            rows = slice(r * CW + lh * DH, r * CW + (lh + 1) * DH)
            perm[rows] = np.arange(g * DH, (g + 1) * DH)
    Wo_p = Wo_[perm, :]

    in_maps = []
    for c in range(N_CORES):
        b, r = c // NGRP, c % NGRP
        gheads = [r + NGRP * lh for lh in range(HPC)]
        col_idx = np.concatenate([np.arange(g * DH, (g + 1) * DH)
                                  for g in gheads])
        # k-side channel values exact in bf16 (small ints / multiples of 128);
        # the slope sits q-side as a constant row, so its single bf16
        # rounding is a benign systematic slope tilt (cancels where masked,
        # proportional to the bias where not).
        ca = np.zeros((2, HPC, NAUG, T), np.float32)
        for lh in range(HPC):
            sl = slopes[gheads[lh]]
            ca[0, lh, 0] = (tok % 128) - 64.0
            ca[0, lh, 1] = 128.0 * np.floor(tok / 128.0)
            ca[0, lh, 2] = 1.0
            ca[1, lh, 0] = sl
            ca[1, lh, 1] = sl
            # +64*sl recenters so max bias (at k=q) is 0: keeps softmax
            # denominators in a range reciprocal_approx_fast handles.
            ca[1, lh, 2] = sl * (64.0 - tok)
        xT_c = np.ascontiguousarray(hs[b].T).astype(BF)
        in_maps.append({
            "xT": xT_c,
            "wq": np.ascontiguousarray(wq_s[:, col_idx]).astype(BF),
            "wk": np.ascontiguousarray(Wk_[:, col_idx]).astype(BF),
            "wv": np.ascontiguousarray(Wv_[:, col_idx]).astype(BF),
            "wo": np.ascontiguousarray(Wo_p[:, r * CW:(r + 1) * CW]).astype(BF),
            "caug": ca.astype(BF), "msk": msk,
        })
    return in_maps


def assemble(results):
    out = np.empty((B, T, HID), np.float32)
    for c in range(N_CORES):
        b, r = c // NGRP, c % NGRP
        out[b, :, r * CW:(r + 1) * CW] = results[c]["outT"].T
    return out


def kernel(hidden_states, attention_mask, Wq, Wk, Wv, Wo):
    nc = build_nc()
    in_maps = make_in_maps(hidden_states, Wq, Wk, Wv, Wo)
    r = run_bass_kernel_spmd(nc, in_maps, core_ids=list(range(N_CORES)))
    return assemble(r.results)
